# revision 16
# baseline (speedup 1.0000x reference)
"""AxialTransformerBlock Trainium2 kernel (8 NeuronCores, SPMD + AllToAll).

Sharding: sequence-parallel over S (512 rows/core) for LN / channel attention /
MLP; head-parallel via AllToAll for temporal causal attention (8 (c,h) pairs
per core over the full sequence), AllToAll back for the output projection.

On-device layout: feature-major residual stream x^T [D, T] so every GEMM uses
fp32r matmuls with no activation transposes. Host pre-transposes x and all
weights, bakes RoPE cos/sin tables (with even/odd de-interleave folded into the
Wq_t/Wk_t column permutation) and the causal / channel block-diagonal masks.
"""

import numpy as np

N_CORES = 8
S, C, D = 4096, 4, 1024
SB = S // N_CORES          # 512 s-rows per core
TL = SB * C                # 2048 local tokens
H_T, HD_T = 16, 64
H_C, HD_C = 4, 256
F_MLP = 4 * D              # 4096
LN_EPS = 1e-5
TC = 512                   # token chunk for phases A/B
NCH = TL // TC             # 4
MC2 = 256                  # MLP second-gemm chunk

_CACHE = {}


def _build_program():
    import concourse.bass as bass
    import concourse.bacc as bacc
    import concourse.tile as tile
    from concourse import mybir

    F32 = mybir.dt.float32
    F32R = mybir.dt.float32r
    AF = mybir.ActivationFunctionType
    OP = mybir.AluOpType
    ts = bass.ts

    nc = bacc.Bacc("TRN2", target_bir_lowering=False, debug=False,
                   num_devices=N_CORES)

    def din(name, shape):
        return nc.dram_tensor(name, list(shape), F32, kind="ExternalInput").ap()

    xT = din("xT", [D, TL])
    wqcT = din("wqcT", [D, D])
    wkcT = din("wkcT", [D, D])
    wvcT = din("wvcT", [D, D])
    wocT = din("wocT", [D, D])
    wqtT = din("wqtT", [D, D])
    wktT = din("wktT", [D, D])
    wvtT = din("wvtT", [D, D])
    wotT = din("wotT", [D, D])
    w1T = din("w1T", [D, F_MLP])
    w2T = din("w2T", [F_MLP, D])
    gb_c = din("gb_c", [D, 2])   # col0 = g, col1 = b
    gb_t = din("gb_t", [D, 2])
    gb_m = din("gb_m", [D, 2])
    b1v = din("b1v", [F_MLP, 1])
    b2v = din("b2v", [D, 1])
    cq_d = din("cq", [128, TC])
    sq_d = din("sq", [128, TC])
    ck_d = din("ck", [128, TC])
    sk_d = din("sk", [128, TC])
    mkc_d = din("mkc", [128, 128])
    mkt_d = din("mkt", [4, 128, TC])
    id_d = din("idm", [128, 128])

    yT = nc.dram_tensor("yT", [D, TL], F32, kind="ExternalOutput").ap()

    # internal DRAM
    import os
    dbg = os.environ.get("KDBG", "0") == "1"
    kindd = "ExternalOutput" if dbg else "Internal"
    x1cm = nc.dram_tensor("x1cm", [D, TL], F32R, kind=kindd).ap()
    x2cm = nc.dram_tensor("x2cm", [D, TL], F32R, kind=kindd).ap()
    hbuf = nc.dram_tensor("hbuf", [F_MLP, TL], F32R, kind=kindd).ap()
    a2aQi = nc.dram_tensor("a2aQi", [8, 8, 64, TC], F32).ap()
    a2aQo = nc.dram_tensor("a2aQo", [8, 8, 64, TC], F32).ap()
    a2aKi = nc.dram_tensor("a2aKi", [8, 8, 64, TC], F32).ap()
    a2aKo = nc.dram_tensor("a2aKo", [8, 8, 64, TC], F32).ap()
    a2aVi = nc.dram_tensor("a2aVi", [8, 8, TC, 64], F32).ap()
    a2aVo = nc.dram_tensor("a2aVo", [8, 8, TC, 64], F32).ap()
    a2aAi = nc.dram_tensor("a2aAi", [8, 8, 64, TC], F32).ap()
    a2aAo = nc.dram_tensor("a2aAo", [8, 8, 64, TC], F32).ap()

    RG = [list(range(N_CORES))]

    def kpe(w):  # [D_in, E] dram -> [p, kt, e] view
        return w.bitcast(F32R).rearrange("(k p) e -> p k e", p=128)

    with tile.TileContext(nc) as tc:
        cst_cm = tc.tile_pool(name="cst", bufs=1)
        cst = cst_cm.__enter__()
        ones1f = cst.tile([128, 1], F32)
        nc.vector.memset(ones1f, 1.0)
        ones1 = ones1f.bitcast(F32R)
        eps1 = cst.tile([1, 1], F32)
        nc.vector.memset(eps1, LN_EPS)
        gbc_sb = cst.tile([128, 8, 2], F32)
        nc.sync.dma_start(out=gbc_sb, in_=gb_c.rearrange("(k p) two -> p k two", p=128))
        gbt_sb = cst.tile([128, 8, 2], F32)
        nc.sync.dma_start(out=gbt_sb, in_=gb_t.rearrange("(k p) two -> p k two", p=128))
        gbm_sb = cst.tile([128, 8, 2], F32)
        nc.sync.dma_start(out=gbm_sb, in_=gb_m.rearrange("(k p) two -> p k two", p=128))
        b1_sb = cst.tile([128, 32], F32)
        nc.sync.dma_start(out=b1_sb, in_=b1v.rearrange("(k p) one -> p (k one)", p=128))
        b2_sb = cst.tile([128, 8], F32)
        nc.sync.dma_start(out=b2_sb, in_=b2v.rearrange("(k p) one -> p (k one)", p=128))

        def layernorm(pool, psum, x_ch, gb_sb, width):
            """x_ch [128, 8, width] f32r -> n_ch same shape; returns n_ch."""
            nkt = 8
            stat_ps = psum.tile([1, width], F32, tag="stat_ps",
                                name="stat_ps", bufs=1)
            stat_ps2 = psum.tile([1, width], F32, tag="stat_ps2",
                                 name="stat_ps2", bufs=1)
            for kt in range(nkt):
                nc.tensor.matmul(stat_ps, ones1, x_ch[:, kt, :],
                                 start=(kt == 0), stop=(kt == nkt - 1))
            for kt in range(nkt):
                xsq = pool.tile([128, width], F32R, tag="ln_xsq", name="xsq")
                nc.scalar.activation(xsq, x_ch[:, kt, :], AF.Square)
                nc.tensor.matmul(stat_ps2, ones1, xsq,
                                 start=(kt == 0), stop=(kt == nkt - 1))
            mu = pool.tile([1, width], F32, tag="ln_mu", name="mu")
            nc.vector.tensor_scalar_mul(mu, stat_ps, 1.0 / D)
            ex2 = pool.tile([1, width], F32, tag="ln_ex2", name="ex2")
            nc.vector.tensor_scalar_mul(ex2, stat_ps2, 1.0 / D)
            var = pool.tile([1, width], F32, tag="ln_var", name="var")
            nc.vector.tensor_tensor(var, mu, mu, OP.mult)
            nc.vector.tensor_tensor(var, ex2, var, OP.subtract)
            sd = pool.tile([1, width], F32, tag="ln_sd", name="sd")
            nc.scalar.activation(sd, var, AF.Sqrt, bias=eps1)
            rs = pool.tile([1, width], F32, tag="ln_rs", name="rs")
            nc.vector.reciprocal(rs, sd)
            bv = pool.tile([1, width], F32, tag="ln_bv", name="bv")
            nc.vector.tensor_tensor(bv, mu, rs, OP.mult)
            ab = pool.tile([128, width], F32R, tag="ln_ab", name="ab")
            nc.gpsimd.partition_broadcast(ab, rs.bitcast(F32R))
            bb = pool.tile([128, width], F32R, tag="ln_bb", name="bb")
            nc.gpsimd.partition_broadcast(bb, bv.bitcast(F32R))
            n_ch = pool.tile([128, 8, width], F32R, tag="ln_out",
                             name="n_ch", bufs=1)
            for kt in range(nkt):
                t1 = pool.tile([128, width], F32R, tag="ln_t1", name="t1")
                nc.vector.tensor_tensor(t1, x_ch[:, kt, :], ab, OP.mult)
                nc.vector.tensor_tensor(t1, t1, bb, OP.subtract)
                nc.vector.tensor_scalar(n_ch[:, kt, :], t1,
                                        gb_sb[:, kt, 0:1], gb_sb[:, kt, 1:2],
                                        OP.mult, OP.add)
            return n_ch

        def proj_fmajor(pool, psum, wdram, n_ch, out_ch, width):
            """out_ch[:, et, :] = W_et^T @ n  (feature-major out)."""
            for et in range(8):
                w_t = pool.tile([128, 8, 128], F32R, tag="wstream", name="w_t")
                nc.sync.dma_start(out=w_t, in_=kpe(wdram)[:, :, ts(et, 128)])
                ps = psum.tile([128, width], F32, tag="ps_proj", name="ps",
                               bufs=1)
                for kt in range(8):
                    nc.tensor.matmul(ps, w_t[:, kt, :], n_ch[:, kt, :],
                                     start=(kt == 0), stop=(kt == 7))
                nc.scalar.activation(out_ch[:, et, :], ps, AF.Copy)

        def proj_tmajor(pool, psum, wdram, n_ch, out_ch, width):
            """V token-major: out_ch [128, width//128, 1024]; k-outer with
            width//128 live psum banks so Wv streams in small tiles."""
            ntt = width // 128
            for ec in range(2):
                psv = [psum.tile([128, 512], F32, tag=f"psv{i}",
                                 name=f"psv{i}", bufs=1) for i in range(ntt)]
                for kt in range(8):
                    w_t = pool.tile([128, 512], F32R, tag="wstreamV",
                                    name="w_tv")
                    nc.sync.dma_start(out=w_t,
                                      in_=kpe(wdram)[:, kt, ts(ec, 512)])
                    for tt_ in range(ntt):
                        nc.tensor.matmul(psv[tt_], n_ch[:, kt, ts(tt_, 128)],
                                         w_t, start=(kt == 0), stop=(kt == 7))
                for tt_ in range(ntt):
                    nc.scalar.activation(out_ch[:, tt_, ts(ec, 512)], psv[tt_],
                                         AF.Copy)

        # ---------------- Phase A: channel attention ----------------
        with (tc.tile_pool(name="pa", bufs=2) as pa,
              tc.tile_pool(name="pa1", bufs=1) as pa1,
              tc.tile_pool(name="pa_ps", bufs=1, space="PSUM") as pa_ps):
            mkc_sb = pa1.tile([128, 128], F32, tag="mkc", name="mkc_sb")
            nc.sync.dma_start(out=mkc_sb, in_=mkc_d)
            id_sb = pa1.tile([128, 128], F32, tag="idm", name="id_sb")
            nc.sync.dma_start(out=id_sb, in_=id_d)
            for ch in range(NCH):
                x_ch = pa1.tile([128, 8, TC], F32R, tag="x_ch", name="x_ch")
                nc.sync.dma_start(
                    out=x_ch,
                    in_=xT.bitcast(F32R).rearrange("(k p) t -> p k t", p=128)[:, :, ts(ch, TC)])
                n_ch = layernorm(pa, pa_ps, x_ch, gbc_sb, TC)
                q_ch = pa1.tile([128, 8, TC], F32R, tag="q_ch", name="q_ch")
                proj_fmajor(pa, pa_ps, wqcT, n_ch, q_ch, TC)
                k_ch = pa1.tile([128, 8, TC], F32R, tag="k_ch", name="k_ch")
                proj_fmajor(pa, pa_ps, wkcT, n_ch, k_ch, TC)
                v_ch = pa1.tile([128, 4, 1024], F32R, tag="v_ch", name="v_ch")
                proj_tmajor(pa, pa_ps, wvcT, n_ch, v_ch, TC)

                # attention (block-diag over 4 channels, s-major tokens)
                pTs = {}
                for h in range(H_C):
                    for qt in range(4):
                        ps_s = pa_ps.tile([128, 128], F32, tag="psv0",
                                          name="ps_s", bufs=1)
                        for hf in range(2):
                            et = h * 2 + hf
                            nc.tensor.matmul(ps_s, q_ch[:, et, ts(qt, 128)],
                                             k_ch[:, et, ts(qt, 128)],
                                             start=(hf == 0), stop=(hf == 1))
                        pe = pa.tile([128, 128], F32, tag="pe", name="pe")
                        nc.scalar.activation(pe, ps_s, AF.Exp, scale=1.0 / 16.0)
                        pm = pa.tile([128, 128], F32, tag="pm", name="pm")
                        den = pa.tile([128, 1], F32, tag="den", name="den")
                        nc.vector.tensor_tensor(pm, pe, mkc_sb, OP.mult)
                        nc.vector.reduce_sum(den, pm, axis=mybir.AxisListType.X)
                        rec = pa.tile([128, 1], F32, tag="rec", name="rec")
                        nc.vector.reciprocal(rec, den)
                        nc.vector.tensor_scalar_mul(pm, pm, rec)
                        ps_t = pa_ps.tile([128, 128], F32, tag="psv1",
                                          name="ps_t", bufs=1)
                        nc.tensor.transpose(ps_t, pm, id_sb)
                        pT = pa1.tile([128, 128], F32R, tag=f"pT_{h}_{qt}",
                                      name=f"pT_{h}_{qt}")
                        nc.scalar.activation(pT, ps_t, AF.Copy)
                        pTs[(h, qt)] = pT
                aT_ch = pa.tile([128, 8, TC], F32R, tag="ln_out",
                                name="aT_ch", bufs=1)
                for es in range(8):
                    ps_av = pa_ps.tile([128, TC], F32, tag="psv2",
                                       name="ps_av", bufs=1)
                    for qt in range(4):
                        nc.tensor.matmul(ps_av[:, ts(qt, 128)],
                                         v_ch[:, qt, ts(es, 128)],
                                         pTs[(es // 2, qt)],
                                         start=True, stop=True)
                    nc.scalar.activation(aT_ch[:, es, :], ps_av, AF.Copy)
                # Wo + residual, write c-major
                for dt in range(8):
                    w_t = pa.tile([128, 8, 128], F32R, tag="wstream", name="w_t")
                    nc.sync.dma_start(out=w_t, in_=kpe(wocT)[:, :, ts(dt, 128)])
                    ps_o = pa_ps.tile([128, TC], F32, tag="psv3", name="ps_o",
                                      bufs=1)
                    for et in range(8):
                        nc.tensor.matmul(ps_o, w_t[:, et, :], aT_ch[:, et, :],
                                         start=(et == 0), stop=(et == 7))
                    x1t = pa.tile([128, 4, 128], F32R, tag="x1t", name="x1t")
                    nc.vector.tensor_tensor(
                        x1t.rearrange("p c s -> p s c"),
                        ps_o.rearrange("p (s c) -> p s c", c=4),
                        x_ch[:, dt, :].rearrange("p (s c) -> p s c", c=4),
                        OP.add)
                    nc.sync.dma_start(
                        out=x1cm[ts(dt, 128), :].rearrange("p (c s) -> p c s", c=4)[:, :, ts(ch, 128)],
                        in_=x1t)

        # ---------------- Phase B: temporal attention ----------------
        with (tc.tile_pool(name="pb", bufs=2) as pb,
              tc.tile_pool(name="pb1", bufs=1) as pb1,
              tc.tile_pool(name="pb_ps", bufs=1, space="PSUM") as pb_ps):
            cq_sb = pb1.tile([128, TC], F32R, tag="cq", name="cq_sb")
            nc.sync.dma_start(out=cq_sb, in_=cq_d.bitcast(F32R))
            sq_sb = pb1.tile([128, TC], F32R, tag="sq", name="sq_sb")
            nc.sync.dma_start(out=sq_sb, in_=sq_d.bitcast(F32R))
            ck_sb = pb1.tile([128, TC], F32R, tag="ck", name="ck_sb")
            nc.sync.dma_start(out=ck_sb, in_=ck_d.bitcast(F32R))
            sk_sb = pb1.tile([128, TC], F32R, tag="sk", name="sk_sb")
            nc.sync.dma_start(out=sk_sb, in_=sk_d.bitcast(F32R))
            mkt_sb = pb1.tile([128, 4, TC], F32R, tag="mkt", name="mkt_sb")
            nc.sync.dma_start(out=mkt_sb,
                              in_=mkt_d.bitcast(F32R).rearrange("k p q -> p k q"))
            for c in range(C):
                x1_ch = pb1.tile([128, 8, TC], F32R, tag="x_ch", name="x1_ch")
                nc.sync.dma_start(
                    out=x1_ch,
                    in_=x1cm.rearrange("(k p) t -> p k t", p=128)[:, :, ts(c, TC)])
                n_ch = layernorm(pb, pb_ps, x1_ch, gbt_sb, TC)
                q_ch = pb1.tile([128, 8, TC], F32R, tag="q_ch", name="q_ch")
                proj_fmajor(pb, pb_ps, wqtT, n_ch, q_ch, TC)
                k_ch = pb1.tile([128, 8, TC], F32R, tag="k_ch", name="k_ch")
                proj_fmajor(pb, pb_ps, wktT, n_ch, k_ch, TC)
                v_ch = pb1.tile([128, 4, 1024], F32R, tag="v_ch", name="v_ch")
                proj_tmajor(pb, pb_ps, wvtT, n_ch, v_ch, TC)
                # RoPE in place on q_ch/k_ch (ev tiles kt, od tiles kt+4)
                for tgt, cos_sb, sin_sb in ((q_ch, cq_sb, sq_sb),
                                            (k_ch, ck_sb, sk_sb)):
                    for pr in range(4):
                        ev = tgt[:, pr, :]
                        od = tgt[:, pr + 4, :]
                        t1 = pb.tile([128, TC], F32R, tag="rp1", name="t1")
                        t2 = pb.tile([128, TC], F32R, tag="rp2", name="t2")
                        t3 = pb.tile([128, TC], F32R, tag="rp3", name="t3")
                        t4 = pb.tile([128, TC], F32R, tag="rp4", name="t4")
                        nc.vector.tensor_tensor(t1, ev, cos_sb, OP.mult)
                        nc.vector.tensor_tensor(t2, ev, sin_sb, OP.mult)
                        nc.vector.tensor_tensor(t3, od, sin_sb, OP.mult)
                        nc.vector.tensor_tensor(t4, od, cos_sb, OP.mult)
                        nc.vector.tensor_tensor(ev, t1, t3, OP.subtract)
                        nc.vector.tensor_tensor(od, t2, t4, OP.add)
                # scatter to A2A send buffers
                for h in range(H_T):
                    g = c * H_T + h
                    j, pl = g // 8, g % 8
                    prow = (h % 4) * 32
                    nc.sync.dma_start(out=a2aQi[j, pl, 0:32, :].bitcast(F32R),
                                      in_=q_ch[prow:prow + 32, h // 4, :])
                    nc.sync.dma_start(out=a2aQi[j, pl, 32:64, :].bitcast(F32R),
                                      in_=q_ch[prow:prow + 32, 4 + h // 4, :])
                    nc.sync.dma_start(out=a2aKi[j, pl, 0:32, :].bitcast(F32R),
                                      in_=k_ch[prow:prow + 32, h // 4, :])
                    nc.sync.dma_start(out=a2aKi[j, pl, 32:64, :].bitcast(F32R),
                                      in_=k_ch[prow:prow + 32, 4 + h // 4, :])
                    nc.sync.dma_start(
                        out=a2aVi[j, pl].bitcast(F32R).rearrange("(tt p) hd -> p tt hd", p=128),
                        in_=v_ch[:, :, ts(h, 64)])
            for src, dst in ((a2aQi, a2aQo), (a2aKi, a2aKo), (a2aVi, a2aVo)):
                nc.gpsimd.collective_compute(
                    "AllToAll", OP.bypass, replica_groups=RG,
                    ins=[src.opt()], outs=[dst.opt()])

            # flash attention per local pair over full S
            for p in range(8):
                kTp = pb1.tile([64, S], F32R, tag="x_ch", name="kTp")
                for src in range(8):
                    nc.sync.dma_start(out=kTp[:, ts(src, TC)],
                                      in_=a2aKo[src, p].bitcast(F32R))
                vp = pb1.tile([128, 32, 65], F32R, tag="q_ch", name="vp")
                for kt in range(32):
                    nc.sync.dma_start(
                        out=vp[:, kt, 0:64],
                        in_=a2aVo[kt // 4, p].bitcast(F32R)[ts(kt % 4, 128), :])
                nc.vector.memset(vp[:, :, 64:65].bitcast(F32), 1.0)
                for qc in range(8):
                    qTp = pb.tile([64, TC], F32R, tag="qTp", name="qTp")
                    nc.sync.dma_start(out=qTp, in_=a2aQo[qc, p].bitcast(F32R))
                    ps_a = pb_ps.tile([128, TC], F32, tag=f"psv{2 + qc % 2}",
                                      name="ps_a", bufs=1)
                    nk = (qc + 1) * 4
                    for kt in range(nk):
                        ps_sc = pb_ps.tile([128, TC], F32, tag=f"psv{kt % 2}",
                                           name="ps_sc", bufs=1)
                        nc.tensor.matmul(ps_sc, kTp[:, ts(kt, 128)], qTp,
                                         start=True, stop=True)
                        pexp = pb.tile([128, TC], F32R, tag="pexp", name="pexp")
                        nc.scalar.activation(pexp, ps_sc, AF.Exp)
                        if kt >= qc * 4:
                            nc.vector.tensor_tensor(pexp, pexp,
                                                    mkt_sb[:, kt - qc * 4, :],
                                                    OP.mult)
                        nc.tensor.matmul(ps_a[0:65, :], vp[:, kt, :], pexp,
                                         start=(kt == 0), stop=(kt == nk - 1))
                    rec1 = pb.tile([1, TC], F32, tag="rec1", name="rec1")
                    nc.vector.reciprocal(rec1, ps_a[64:65, :])
                    rb = pb.tile([64, TC], F32R, tag="rb", name="rb")
                    nc.gpsimd.partition_broadcast(rb, rec1.bitcast(F32R))
                    aT = pb.tile([64, TC], F32, tag="aT", name="aT")
                    nc.vector.tensor_tensor(aT, ps_a[0:64, :], rb, OP.mult)
                    nc.sync.dma_start(out=a2aAi[qc, p], in_=aT)
            nc.gpsimd.collective_compute(
                "AllToAll", OP.bypass, replica_groups=RG,
                ins=[a2aAi.opt()], outs=[a2aAo.opt()])

            # Wo_t + residual per channel
            for c in range(C):
                for dt in range(8):
                    w_t = pb.tile([128, 8, 128], F32R, tag="wstream", name="w_t")
                    nc.sync.dma_start(out=w_t, in_=kpe(wotT)[:, :, ts(dt, 128)])
                    ps_o = pb_ps.tile([128, TC], F32, tag="ps_proj",
                                      name="ps_o", bufs=1)
                    for et in range(8):
                        g0 = c * H_T + et * 2
                        rhsA = pb.tile([128, TC], F32R, tag="rhsA", name="rhsA")
                        nc.sync.dma_start(
                            out=rhsA,
                            in_=a2aAo[g0 // 8, g0 % 8:g0 % 8 + 2].bitcast(F32R).rearrange("a r q -> (a r) q"))
                        nc.tensor.matmul(ps_o, w_t[:, et, :], rhsA,
                                         start=(et == 0), stop=(et == 7))
                    x1c = pb.tile([128, TC], F32R, tag="x1c", name="x1c")
                    nc.sync.dma_start(out=x1c, in_=x1cm[ts(dt, 128), ts(c, TC)])
                    x2t = pb.tile([128, TC], F32R, tag="x2t", name="x2t")
                    nc.vector.tensor_tensor(x2t, ps_o, x1c, OP.add)
                    nc.sync.dma_start(out=x2cm[ts(dt, 128), ts(c, TC)], in_=x2t)

        # ---------------- Phase C: MLP ----------------
        with (tc.tile_pool(name="pm1", bufs=2) as pm1,
              tc.tile_pool(name="pm1b", bufs=1) as pm1b,
              tc.tile_pool(name="pm1_ps", bufs=2, space="PSUM") as pm1_ps):
            n_m = pm1b.tile([128, 8, TL], F32R, tag="n_m", name="n_m")
            for ch in range(NCH):
                x2_ch = pm1.tile([128, 8, TC], F32R, tag="x_ch", name="x2_ch",
                                 bufs=1)
                nc.sync.dma_start(
                    out=x2_ch,
                    in_=x2cm.rearrange("(k p) t -> p k t", p=128)[:, :, ts(ch, TC)])
                nloc = layernorm(pm1, pm1_ps, x2_ch, gbm_sb, TC)
                for kt in range(8):
                    nc.vector.tensor_copy(out=n_m[:, kt, ts(ch, TC)],
                                          in_=nloc[:, kt, :])
            for ft in range(32):
                w_t = pm1.tile([128, 8, 128], F32R, tag="wstream", name="w_t")
                nc.sync.dma_start(out=w_t, in_=kpe(w1T)[:, :, ts(ft, 128)])
                for ch in range(NCH):
                    ps1 = pm1_ps.tile([128, TC], F32, tag="ps_m1", name="ps1")
                    for kt in range(8):
                        nc.tensor.matmul(ps1, w_t[:, kt, :],
                                         n_m[:, kt, ts(ch, TC)],
                                         start=(kt == 0), stop=(kt == 7))
                    hft = pm1.tile([128, TC], F32R, tag="hft", name="hft")
                    nc.scalar.activation(hft, ps1, AF.Relu,
                                         bias=b1_sb[:, ft:ft + 1])
                    nc.sync.dma_start(out=hbuf[ts(ft, 128), ts(ch, TC)], in_=hft)

        with (tc.tile_pool(name="pm2", bufs=3) as pm2,
              tc.tile_pool(name="pm2b", bufs=1) as pm2b,
              tc.tile_pool(name="pm2_ps", bufs=1, space="PSUM") as pm2_ps):
            w2_sb = pm2b.tile([128, 32, D], F32R, tag="w2_sb", name="w2_sb")
            nc.sync.dma_start(out=w2_sb, in_=kpe(w2T))
            for c2 in range(TL // MC2):
                psD = []
                for i in range(8):
                    pd = pm2_ps.tile([128, MC2], F32, tag=f"ps_m2_{i}",
                                     name=f"psD{i}", bufs=1)
                    psD.append(pd)
                for ft in range(32):
                    hft2 = pm2.tile([128, MC2], F32R, tag="hstream", name="hft2")
                    nc.sync.dma_start(out=hft2,
                                      in_=hbuf[ts(ft, 128), ts(c2, MC2)])
                    for dt in range(8):
                        nc.tensor.matmul(
                            psD[dt],
                            w2_sb[:, ft, ts(dt, 128)], hft2,
                            start=(ft == 0), stop=(ft == 31))
                x2_c2 = pm2.tile([128, 8, MC2], F32R, tag="x2_c2", name="x2_c2",
                                 bufs=2)
                nc.sync.dma_start(
                    out=x2_c2,
                    in_=x2cm.rearrange("(k p) t -> p k t", p=128)[:, :, ts(c2, MC2)])
                for dt in range(8):
                    yt_t = pm2.tile([128, MC2], F32R, tag="ytt", name="yt_t")
                    nc.vector.tensor_tensor(yt_t, psD[dt],
                                            x2_c2[:, dt, :], OP.add)
                    nc.vector.tensor_scalar_add(yt_t, yt_t, b2_sb[:, dt:dt + 1])
                    nc.sync.dma_start(out=yT[ts(dt, 128), ts(c2, MC2)].bitcast(F32R),
                                      in_=yt_t)
        cst_cm.__exit__(None, None, None)

    nc.finalize()
    in_names = ["xT", "wqcT", "wkcT", "wvcT", "wocT", "wqtT", "wktT", "wvtT",
                "wotT", "w1T", "w2T", "gb_c", "gb_t", "gb_m", "b1v", "b2v",
                "cq", "sq", "ck", "sk", "mkc", "mkt", "idm"]
    return nc, in_names


def _host_prep(inputs):
    """Build per-core in_maps from full inputs."""
    x = np.asarray(inputs["x"], np.float32)
    positions = np.asarray(inputs["positions"]).astype(np.int64)

    def T(a):
        return np.ascontiguousarray(np.asarray(a, np.float32).T)

    # temporal Q/K column permutation: [all evens (h-major, freq), all odds]
    perm = np.zeros(D, np.int64)
    for h in range(H_T):
        for i in range(32):
            perm[h * 32 + i] = h * 64 + 2 * i
            perm[512 + h * 32 + i] = h * 64 + 2 * i + 1
    wqtT = np.ascontiguousarray(T(inputs["Wq_t"])[:, perm])
    wktT = np.ascontiguousarray(T(inputs["Wk_t"])[:, perm])

    def gb(g, b):
        return np.ascontiguousarray(
            np.stack([np.asarray(g, np.float32), np.asarray(b, np.float32)],
                     axis=1))

    shared = {
        "wqcT": T(inputs["Wq_c"]), "wkcT": T(inputs["Wk_c"]),
        "wvcT": T(inputs["Wv_c"]), "wocT": T(inputs["Wo_c"]),
        "wqtT": wqtT, "wktT": wktT,
        "wvtT": T(inputs["Wv_t"]), "wotT": T(inputs["Wo_t"]),
        "w1T": T(inputs["W1"]), "w2T": T(inputs["W2"]),
        "gb_c": gb(inputs["g_c"], inputs["b_c"]),
        "gb_t": gb(inputs["g_t"], inputs["b_t"]),
        "gb_m": gb(inputs["g_m"], inputs["b_m"]),
        "b1v": np.asarray(inputs["b1"], np.float32).reshape(F_MLP, 1),
        "b2v": np.asarray(inputs["b2"], np.float32).reshape(D, 1),
    }
    # channel block-diag mask (tokens s-major, groups of 4)
    idx = np.arange(128)
    shared["mkc"] = (idx[:, None] // 4 == idx[None, :] // 4).astype(np.float32)
    # temporal causal masks for the 4 diagonal k-tiles of a 512 q-chunk
    mkt = np.zeros((4, 128, TC), np.float32)
    dq = np.arange(TC)
    dk = np.arange(128)
    for kt in range(4):
        mkt[kt] = (dq[None, :] >= kt * 128 + dk[:, None]).astype(np.float32)
    shared["mkt"] = mkt
    shared["idm"] = np.eye(128, dtype=np.float32)

    inv_freq = (10000.0 ** (-np.arange(32, dtype=np.float64) * 2 / HD_T))
    in_maps = []
    for i in range(N_CORES):
        m = dict(shared)
        xs = x[i * SB:(i + 1) * SB].reshape(TL, D)
        m["xT"] = np.ascontiguousarray(xs.T)
        pos = positions[i * SB:(i + 1) * SB].astype(np.float64)
        ang = pos[:, None] * inv_freq[None, :]          # [512, 32]
        cosT = np.cos(ang).T.astype(np.float32)         # [32, 512]
        sinT = np.sin(ang).T.astype(np.float32)
        c4 = np.tile(cosT, (4, 1))
        s4 = np.tile(sinT, (4, 1))
        m["cq"] = np.ascontiguousarray(c4 * 0.125)
        m["sq"] = np.ascontiguousarray(s4 * 0.125)
        m["ck"] = np.ascontiguousarray(c4)
        m["sk"] = np.ascontiguousarray(s4)
        in_maps.append(m)
    return in_maps


def _run(inputs, trace=False):
    from concourse.bass_utils import run_bass_kernel_spmd
    if "prog" not in _CACHE:
        _CACHE["prog"] = _build_program()
    nc, in_names = _CACHE["prog"]
    in_maps = _host_prep(inputs)
    for m in in_maps:
        for k in list(m.keys()):
            assert k in in_names, k
    res = run_bass_kernel_spmd(nc, in_maps, core_ids=list(range(N_CORES)),
                               trace=trace)
    out = np.zeros((S, C, D), np.float32)
    for i in range(N_CORES):
        yT = res.results[i]["yT"]                        # [1024, 2048] c-major
        yi = yT.T.reshape(C, SB, D)                      # [c, s, d]
        out[i * SB:(i + 1) * SB] = yi.transpose(1, 0, 2)
    return out, res


def kernel(**inputs) -> np.ndarray:
    out, _ = _run(inputs, trace=False)
    return out
